# revision 51
# baseline (speedup 1.0000x reference)
"""
Trainium2 Bass kernel v2 for nn_Attention (dense transformer attention block).

Sharding (8 cores): core c -> batch b=c//4, head-group g=c%4 (4 heads = 256
dims).  Host sums the 4 partial proj outputs per batch (Megatron-style).

Structure: fully chunk-streamed over 8 x 256-token chunks.  Per chunk:
Q proj (chunk tokens) -> K proj -> V proj -> attention (S/exp/AV k-tile
streamed) -> normalize -> PE transpose -> output proj -> y store.  This
spreads the ACT-engine exp work across the whole kernel so it overlaps the
PE-heavy projections, instead of phase-separating them.

Datapath (all matmuls bf16, fp32 PSUM accumulate):
  - S^T per k-tile: one 2-bank psum [128, 4x256] holds all 4 heads; ONE exp
    instruction per k-tile ([128, 4, w] strided) -> pt bf16 [128, 1024];
    diagonal causal mask via one [128, 4, 128] DVE multiply.
  - AV transposed: out[q=128, 65] per (q-sub, head) accumulating over
    k-tiles; lhsT = pt slice, rhs = VH[k-tile] (64 V cols + ones col ->
    softmax denominator free in col 64).  Fully-masked (q-sub, k-tile)
    pairs are skipped.  Accumulators: 2 psum banks [128, 260] (head-pair
    major, both q-subs packed; single start/stop per bank).
  - Normalize with per-partition reciprocal (q on partitions), write bf16,
    PE-transpose (identity matmul) back to [d, q] for the proj lhsT.

PSUM budget (8 banks): sp 2x2 + av 2x1 + pp 2x1.
"""

import sys
import numpy as np

for _p in ("/opt/trn_rl_repo",):
    if _p not in sys.path:
        sys.path.insert(0, _p)

import concourse.bass as bass
import concourse.bacc as bacc
import concourse.mybir as mybir
import concourse.tile as tile
from concourse.bass import ts
from concourse.bass_utils import run_bass_kernel_spmd

F32 = mybir.dt.float32
BF16 = mybir.dt.bfloat16
EXP = mybir.ActivationFunctionType.Exp

HID = 1024          # hidden dim
DS = 256            # per-core dim slice (4 heads x 64)
NT = 2048           # tokens per batch
HD = 64             # head dim
SCALE = HD ** -0.5
NKT = HID // 128    # hidden contraction tiles
NTOK = NT // 128    # token tiles of 128
NCH = NT // 256     # 256-wide token chunks

_NC_CACHE = {}
LABELS = {}


def _mm(nc, out, lhsT, rhs, start, stop, label=""):
    r = nc.tensor.matmul(out, lhsT, rhs, start=start, stop=stop)
    if label:
        try:
            LABELS[r.ins.name] = label
        except AttributeError:
            pass
    return r


def _build_nc(reps=1, upto="full", loop=None):
    """loop=N wraps one rep body in a hardware For_i(0, N) loop — used only
    for slope timing (amplifies signal without growing the NEFF)."""
    from contextlib import ExitStack, nullcontext

    nc = bacc.Bacc(num_swdge_queues=4)
    xqT = nc.declare_dram_parameter("xqT", [HID, NT], BF16, isOutput=False)
    xkT = nc.declare_dram_parameter("xkT", [HID, NT], BF16, isOutput=False)
    vT = nc.declare_dram_parameter("vT", [HID, NT], BF16, isOutput=False)
    wqT = nc.declare_dram_parameter("wqT", [128, NKT, DS], BF16, isOutput=False)
    wkT = nc.declare_dram_parameter("wkT", [128, NKT, DS], BF16, isOutput=False)
    wvT = nc.declare_dram_parameter("wvT", [128, NKT, DS], BF16, isOutput=False)
    wqb = nc.declare_dram_parameter("wqb", [128, 2], F32, isOutput=False)
    wkb = nc.declare_dram_parameter("wkb", [128, 2], F32, isOutput=False)
    projT = nc.declare_dram_parameter("projT", [128, 2, HID], BF16,
                                      isOutput=False)
    trimask4 = nc.declare_dram_parameter("trimask4", [128, 512], BF16,
                                         isOutput=False)
    ident = nc.declare_dram_parameter("ident", [128, 128], BF16,
                                      isOutput=False)
    y = nc.declare_dram_parameter("y", [NT, HID], BF16, isOutput=True)

    with tile.TileContext(nc) as tc, ExitStack() as ctx:
        ctx.enter_context(nc.allow_low_precision(
            reason="bf16 matmul datapath by design; fp32 psum accumulate"))
        pers = ctx.enter_context(tc.tile_pool(name="pers", bufs=1))

        KHT = [pers.tile([128, NT], BF16, tag=f"kht{i}", name=f"kht{i}")
               for i in range(2)]
        # Per-head zero-padded Q tiles (double-buffered by chunk parity).
        # bf16 matmuls with operands at base partition 64 fail at runtime on
        # this stack, so S uses K=128 from base partition 0 with the other
        # head's partitions zeroed in the rhs; the pad halves are zeroed once
        # here and never rewritten (drains only touch the data half).
        QHP = [[pers.tile([128, 256], BF16, tag=f"qh{p}{h}", name=f"qh{p}{h}")
                for h in range(4)] for p in range(2)]
        for p in range(2):
            for h in range(4):
                sub = h % 2
                nc.vector.memset(
                    QHP[p][h][64 * (1 - sub):64 * (1 - sub) + 64, :], 0.0)
        VH = [pers.tile([128, 4 * 65], BF16, tag=f"vh{m}", name=f"vh{m}")
              for m in range(NTOK)]

        wq_s = pers.tile([128, NKT, DS], BF16, tag="wq")
        wk_s = pers.tile([128, NKT, DS], BF16, tag="wk")
        wv_s = pers.tile([128, NKT, DS], BF16, tag="wv")
        pj_s = pers.tile([128, 2, HID], BF16, tag="pj")
        mk_s = pers.tile([128, 512], BF16, tag="mask")
        id_s = pers.tile([128, 128], BF16, tag="ident")
        qb_s = pers.tile([128, 2], F32, tag="wqb")
        kb_s = pers.tile([128, 2], F32, tag="wkb")

        # loop-timing mode: ch7's normalized xh lands in persistent tiles so
        # the NEXT For_i iteration can drain the tail overlapped with its
        # prologue (every iteration computes identical values, so the final
        # iteration's undrained tail leaves the correct y rows from the
        # previous drain).  Zeroed once so iteration 0's drain is finite.
        XH_P = None
        if loop is not None:
            XH_P = [[pers.tile([128, 128], BF16, tag=f"xhp{j}{hp}",
                               name=f"xhp{j}{hp}")
                     for hp in range(2)] for j in range(2)]
            for j in range(2):
                for hp in range(2):
                    nc.vector.memset(XH_P[j][hp][:], 0.0)

        for m in range(NTOK):
            vh3 = VH[m].rearrange("p (h w) -> p h w", w=65)
            nc.vector.memset(vh3[:, :, 64:65], 1.0)

        psp = ctx.enter_context(
            tc.tile_pool(name="psp", bufs=1, space=bass.MemorySpace.PSUM))
        xsp = ctx.enter_context(tc.tile_pool(name="xsp", bufs=1))
        qhp = ctx.enter_context(tc.tile_pool(name="qhp", bufs=8))
        ptp = ctx.enter_context(tc.tile_pool(name="ptp", bufs=18))
        asb = ctx.enter_context(tc.tile_pool(name="asb", bufs=4))
        xhp = ctx.enter_context(tc.tile_pool(name="xhp", bufs=4))
        avn = ctx.enter_context(tc.tile_pool(name="avn", bufs=4))
        ysb = ctx.enter_context(tc.tile_pool(name="ysb", bufs=2))

        # weight DMA order: what chunk 0 needs first (wq, qb) precedes the
        # bulk x streams; the rest rides behind xq
        nc.sync.dma_start(wq_s[:], wqT[:])
        nc.sync.dma_start(qb_s[:], wqb[:])

        first = True
        x3 = [xqT.rearrange("(kt p) n -> p kt n", p=128),
              xkT.rearrange("(kt p) n -> p kt n", p=128),
              vT.rearrange("(kt p) n -> p kt n", p=128)]
        XLOOK = 3                  # chunk prefetch depth
        _pending = {}
        _total_ch = reps * NCH

        def _ensure_load(g):
            if g in _pending or g >= _total_ch:
                return
            cg = g % NCH
            tls = []
            for ti, tag in enumerate(("xq", "xk", "xv")):
                tl = xsp.tile([128, NKT, 256], BF16, tag=tag, bufs=XLOOK + 1,
                              name=tag)
                nc.sync.dma_start(tl[:], x3[ti][:, :, ts(cg, 256)])
                tls.append(tl)
            _pending[g] = tls

        if loop is not None:
            # hardware-loop timing mode: weights load once before the loop
            assert reps == 1
            nc.sync.dma_start(wk_s[:], wkT[:])
            nc.sync.dma_start(kb_s[:], wkb[:])
            nc.sync.dma_start(wv_s[:], wvT[:])
            nc.sync.dma_start(mk_s[:], trimask4[:])
            nc.sync.dma_start(id_s[:], ident[:])
            nc.sync.dma_start(pj_s[:], projT[:])
            first = False
        tail = None
        for _rep in range(reps):
            _loop_cm = tc.For_i(0, loop, 1) if loop is not None else None
            if _loop_cm is not None:
                _loop_cm.__enter__()
            # per-chunk x streaming: one [128, NKT, 256] tile per (tensor,
            # chunk), prefetched XLOOK chunks ahead.  Rotating bufs give
            # cross-rep overlap (next rep's chunk-0 x loads while this rep
            # finishes) and keep SBUF small.
            if first:
                _ensure_load(_rep * NCH)
                nc.sync.dma_start(wk_s[:], wkT[:])
                nc.sync.dma_start(kb_s[:], wkb[:])
                nc.sync.dma_start(wv_s[:], wvT[:])
                nc.sync.dma_start(mk_s[:], trimask4[:])
                nc.sync.dma_start(id_s[:], ident[:])
                nc.sync.dma_start(pj_s[:], projT[:])
                first = False

            def tail_units(xhs, ch_t):
                # transpose / proj / store for a chunk whose normalized xh
                # tiles are ready.  Generator: driven interleaved into the
                # NEXT chunk's attention stream so PE keeps queued work while
                # ACT catches up on exp.
                tps = []
                for j in range(2):
                    tp = psp.tile([128, 256], BF16, tag="av", bufs=2,
                                  name="tp")
                    for hp in range(2):
                        nc.tensor.transpose(tp[:, ts(hp, 128)], xhs[j][hp][:],
                                            id_s[:])
                    tps.append(tp)
                yield
                for j in range(2):
                    m = 2 * ch_t + j
                    at = avn.tile([128, 256], BF16, tag="avn", name="avn")
                    nc.vector.tensor_copy(at[:], tps[j][:])
                    ys = ysb.tile([128, HID], BF16, tag="ys", name="ys")
                    for n2 in range(2):
                        yp = psp.tile([128, 512], F32, tag="pp", bufs=2,
                                      name="yp")
                        for hp in range(2):
                            _mm(nc, yp[:], at[:, ts(hp, 128)],
                                pj_s[:, hp, ts(n2, 512)],
                                start=(hp == 0), stop=(hp == 1),
                                label="proj")
                        nc.vector.tensor_copy(ys[:, ts(n2, 512)], yp[:])
                        yield
                    nc.sync.dma_start(y[ts(m, 128), :], ys[:])

            def proj_units(gp):
                # Q/K/V projections for global chunk gp.  Generator yielding
                # after each contraction step (~2 matmuls); driven interleaved
                # into the PREVIOUS chunk's attention.
                cp = gp % NCH
                for la in range(XLOOK + 1):
                    _ensure_load(gp + la)
                xq_t, xk_t, xv_t = _pending.pop(gp)
                qhn = QHP[cp % 2]
                ps = psp.tile([128, 512], F32, tag="pp", bufs=2, name="psQ")
                for kt in range(NKT):
                    for m in range(2):
                        _mm(nc, ps[:, ts(m, 256)], wq_s[:, kt, ts(m, 128)],
                            xq_t[:, kt, :],
                            start=(kt == 0 and m == 0),
                            stop=(kt == NKT - 1 and m == 1),
                            label="Qproj")
                    yield
                # qh bias-add on DVE (not ACT): keeps ACT exclusively on exp
                # so the exp stream never falls behind the PE's S matmuls
                for h in range(4):
                    m, sub = divmod(h, 2)
                    nc.vector.tensor_scalar_add(
                        qhn[h][64 * sub:64 * sub + 64, :],
                        ps[64 * sub:64 * sub + 64, ts(m, 256)],
                        qb_s[64 * sub:64 * sub + 64, m:m + 1])
                yield
                ps = psp.tile([128, 512], F32, tag="pp", bufs=2, name="psK")
                for kt in range(NKT):
                    for m in range(2):
                        _mm(nc, ps[:, ts(m, 256)], wk_s[:, kt, ts(m, 128)],
                            xk_t[:, kt, :],
                            start=(kt == 0 and m == 0),
                            stop=(kt == NKT - 1 and m == 1),
                            label="Kproj")
                    yield
                for m in range(2):
                    nc.vector.tensor_scalar_add(
                        KHT[m][:, ts(cp, 256)], ps[:, ts(m, 256)],
                        kb_s[:, m:m + 1])
                yield
                # wv_b is folded into the host epilogue (sum p = 1 after
                # normalize, so xh += wv_b  =>  y += wv_b @ proj_w.T)
                ps = psp.tile([128, 512], F32, tag="pp", bufs=2, name="psV")
                for kt in range(NKT):
                    for m2 in range(2):
                        _mm(nc, ps[:, ts(m2, 256)],
                            xv_t[:, kt, ts(m2, 128)],
                            wv_s[:, kt, :],
                            start=(kt == 0 and m2 == 0),
                            stop=(kt == NKT - 1 and m2 == 1),
                            label="Vproj")
                    yield
                for m2 in range(2):
                    vh3 = VH[2 * cp + m2].rearrange("p (h w) -> p h w", w=65)
                    nc.vector.tensor_copy(
                        vh3[:, :, 0:64],
                        ps[:, ts(m2, 256)].rearrange("p (h w) -> p h w", w=64))

            if _rep == 0:
                # prologue: in loop mode, drain the previous iteration's
                # tail first -- it has no x dependence, so its PE/DVE/DMA
                # work fills the chunk-0 x DMA wait; then chunk 0's
                # projections run standalone
                if loop is not None:
                    for _ in tail_units(XH_P, NCH - 1):
                        pass
                for _ in proj_units(_rep * NCH):
                    pass

            for ch in range(NCH):
                g = _rep * NCH + ch
                qh = QHP[ch % 2]
                # interleave units: next chunk's projections first (so the
                # qh/kht DVE writes the next chunk's S(0) needs queue ahead
                # of the tail's copies), then the previous chunk's tail
                units = []
                if g + 1 < _total_ch:
                    units.append(proj_units(g + 1))
                    # a few proj matmuls ahead of the tail's PE transposes:
                    # the transposes wait on the previous chunk's norm (DVE),
                    # and PE is in-order, so give DVE a head start
                    for _ in range(3):
                        try:
                            next(units[0])
                        except StopIteration:
                            break
                if tail is not None:
                    tg = tail_units(*tail)
                    next(tg)  # allocate tp psum tiles before av accumulators
                    units.append(tg)
                    tail = None

                def step(n):
                    for _ in range(n):
                        if not units:
                            return
                        try:
                            next(units[0])
                        except StopIteration:
                            units.pop(0)

                # ---- attention: S/exp/AV streamed over k-tiles, with a
                # one-step skew so AV(i) is emitted after S(i+1); ~31
                # interleave units paced evenly across the k-tiles ----
                nkt = 2 * ch + 2
                NUNITS = 31
                _done = [0]

                def pace(i):
                    tgt = ((i + 1) * NUNITS) // nkt
                    step(tgt - _done[0])
                    _done[0] = tgt
                av = [psp.tile([128, 260], F32, tag="av", bufs=2,
                               name=f"av{hp}") for hp in range(2)]
                pts = []

                def emit_av(i):
                    cs = max(0, 128 * (i - 2 * ch))
                    for j in range(2):
                        if i > 2 * ch + j:
                            continue
                        off = 128 * j - cs
                        for h in range(4):
                            hp, s2 = divmod(h, 2)
                            _mm(nc,
                                av[hp][:, 130 * j + 65 * s2:
                                       130 * j + 65 * s2 + 65],
                                pts[i][:, 256 * h + off:256 * h + off + 128],
                                VH[i][:, 65 * h:65 * h + 65],
                                start=(i == 0 and j == 0 and s2 == 0),
                                stop=(i == 2 * ch + 1 and j == 1
                                      and s2 == 1), label="AV")

                for i in range(nkt):
                    pace(i)  # queue PE work ahead of S so ACT keeps pace
                    d = i - 2 * ch
                    cs = 128 * d if d >= 0 else 0
                    w = 256 - cs
                    sp = psp.tile([128, 1024], F32, tag="sp", bufs=2,
                                  name="sp")
                    for h in range(4):
                        _mm(nc, sp[:, 256 * h:256 * h + w],
                            KHT[h // 2][:, ts(i, 128)], qh[h][:, cs:256],
                            start=(h % 2 == 0), stop=(h % 2 == 1), label="S")
                    pt = ptp.tile([128, 1024], BF16, tag="pt", name="pt")
                    sp4 = sp.rearrange("p (h w) -> p h w", w=256)
                    pt4 = pt.rearrange("p (h w) -> p h w", w=256)
                    nc.scalar.activation(pt4[:, :, 0:w], sp4[:, :, 0:w],
                                         EXP, scale=SCALE)
                    if d >= 0:
                        nc.vector.tensor_mul(
                            pt4[:, :, 0:128], pt4[:, :, 0:128],
                            mk_s[:].rearrange("p (h w) -> p h w", w=128))
                    pts.append(pt)
                    if i > 2:
                        emit_av(i - 3)
                for ii in range(max(0, nkt - 3), nkt):
                    step(2)  # keep PE fed while the last exps finish
                    emit_av(ii)
                step(10 ** 6)  # drain remaining interleave units

                # ---- normalization (DVE): overlaps next chunk's attention
                xhs = []
                for j in range(2):
                    xhj = []
                    for hp in range(2):
                        av4 = av[hp].rearrange("p (j s w) -> p j s w",
                                               j=2, s=2)
                        rec2 = asb.tile([128, 2], F32, tag="rec", name="rec")
                        nc.vector.reciprocal(
                            rec2[:].rearrange("p (s w) -> p s w", w=1),
                            av4[:, j, :, 64:65])
                        if loop is not None and ch == NCH - 1:
                            # persistent tiles: drained at the start of the
                            # NEXT For_i iteration (values are identical
                            # every iteration, so the final y rows are
                            # already correct from the previous drain)
                            xh = XH_P[j][hp]
                        else:
                            xh = xhp.tile([128, 128], BF16, tag="xh", bufs=8,
                                          name="xh")
                        for s2 in range(2):
                            nc.vector.tensor_scalar_mul(
                                xh[:, 64 * s2:64 * s2 + 64],
                                av4[:, j, s2, 0:64], rec2[:, s2:s2 + 1])
                        xhj.append(xh)
                    xhs.append(xhj)
                tail = (xhs, ch)
            if loop is not None:
                # ch7's tail drains at the next iteration's prologue
                tail = None
            if _loop_cm is not None:
                _loop_cm.__exit__(None, None, None)
        if tail is not None:
            for _ in tail_units(*tail):
                pass

    nc.compile()
    return nc


def _get_nc():
    if "nc" not in _NC_CACHE:
        _NC_CACHE["nc"] = _build_nc()
    return _NC_CACHE["nc"]


def make_in_maps(q, k, v, qpos, kpos, mask, wq_w, wq_b, wk_w, wk_b, wv_w, wv_b,
                 proj_w, proj_b):
    f32 = np.float32
    bf16 = mybir.dt.np(BF16)
    q = np.asarray(q, f32); k = np.asarray(k, f32); v = np.asarray(v, f32)
    qpos = np.asarray(qpos, f32); kpos = np.asarray(kpos, f32)
    wq_w = np.asarray(wq_w, f32); wk_w = np.asarray(wk_w, f32)
    wv_w = np.asarray(wv_w, f32); proj_w = np.asarray(proj_w, f32)
    wq_b = np.asarray(wq_b, f32); wk_b = np.asarray(wk_b, f32)
    wv_b = np.asarray(wv_b, f32)

    m2 = np.asarray(mask).reshape(NT, NT)
    # pt layout is [k_local, q_local]; valid (unmasked) = 1.0
    pat = (~m2[0:128, 0:128]).astype(f32).T
    trimask4 = np.concatenate([pat] * 4, axis=1).astype(bf16)
    ident = np.eye(128, dtype=f32).astype(bf16)

    actT = {}
    for b in range(2):
        actT[("xq", b)] = np.ascontiguousarray((q[b] + qpos[b]).T).astype(bf16)
        actT[("xk", b)] = np.ascontiguousarray((k[b] + kpos[b]).T).astype(bf16)
        actT[("v", b)] = np.ascontiguousarray(v[b].T).astype(bf16)

    in_maps = []
    for c in range(8):
        b, g = divmod(c, 4)
        sl = slice(DS * g, DS * (g + 1))
        in_maps.append({
            "xqT": actT[("xq", b)], "xkT": actT[("xk", b)],
            "vT": actT[("v", b)],
            "wqT": np.ascontiguousarray(
                wq_w[sl, :].T.reshape(NKT, 128, DS).transpose(1, 0, 2)
            ).astype(bf16),
            "wkT": np.ascontiguousarray(
                wk_w[sl, :].T.reshape(NKT, 128, DS).transpose(1, 0, 2)
            ).astype(bf16),
            "wvT": np.ascontiguousarray(
                wv_w[sl, :].T.reshape(NKT, 128, DS).transpose(1, 0, 2)
            ).astype(bf16),
            "wqb": np.ascontiguousarray(wq_b[sl].reshape(2, 128).T),
            "wkb": np.ascontiguousarray(wk_b[sl].reshape(2, 128).T),
            "projT": np.ascontiguousarray(
                proj_w[:, sl].T.reshape(2, 128, HID).transpose(1, 0, 2)
            ).astype(bf16),
            "trimask4": trimask4,
            "ident": ident,
        })
    return in_maps


def kernel(q, k, v, qpos, kpos, mask, wq_w, wq_b, wk_w, wk_b, wv_w, wv_b,
           proj_w, proj_b, _trace=False):
    nc = _get_nc()
    in_maps = make_in_maps(q, k, v, qpos, kpos, mask, wq_w, wq_b, wk_w, wk_b,
                           wv_w, wv_b, proj_w, proj_b)
    res = run_bass_kernel_spmd(nc, in_maps, list(range(8)), trace=_trace)
    if _trace:
        kernel._last_results = res
    out = np.zeros((2, NT, HID), np.float32)
    for c in range(8):
        out[c // 4] += np.asarray(res.results[c]["y"], np.float32)
    # V-bias epilogue: after softmax-normalization sum(p)=1, so the V bias
    # adds wv_b to every attention output row => y += wv_b @ proj_w.T.
    epi = (np.asarray(wv_b, np.float64) @ np.asarray(proj_w, np.float64).T
           + np.asarray(proj_b, np.float64))
    out += epi.astype(np.float32)[None, None, :]
    return out



# revision 52
# speedup vs baseline: 1.8531x; 1.8531x over previous
"""
Trainium2 Bass kernel v2 for nn_Attention (dense transformer attention block).

Sharding (8 cores): core c -> batch b=c//4, head-group g=c%4 (4 heads = 256
dims).  Host sums the 4 partial proj outputs per batch (Megatron-style).

Structure: fully chunk-streamed over 8 x 256-token chunks.  Per chunk:
Q proj (chunk tokens) -> K proj -> V proj -> attention (S/exp/AV k-tile
streamed) -> normalize -> PE transpose -> output proj -> y store.  This
spreads the ACT-engine exp work across the whole kernel so it overlaps the
PE-heavy projections, instead of phase-separating them.

Datapath (all matmuls bf16, fp32 PSUM accumulate):
  - S^T per k-tile: one 2-bank psum [128, 4x256] holds all 4 heads; ONE exp
    instruction per k-tile ([128, 4, w] strided) -> pt bf16 [128, 1024];
    diagonal causal mask via one [128, 4, 128] DVE multiply.
  - AV transposed: out[q=128, 65] per (q-sub, head) accumulating over
    k-tiles; lhsT = pt slice, rhs = VH[k-tile] (64 V cols + ones col ->
    softmax denominator free in col 64).  Fully-masked (q-sub, k-tile)
    pairs are skipped.  Accumulators: 2 psum banks [128, 260] (head-pair
    major, both q-subs packed; single start/stop per bank).
  - Normalize with per-partition reciprocal (q on partitions), write bf16,
    PE-transpose (identity matmul) back to [d, q] for the proj lhsT.

PSUM budget (8 banks): sp 2x2 + av 2x1 + pp 2x1.
"""

import sys
import numpy as np

for _p in ("/opt/trn_rl_repo",):
    if _p not in sys.path:
        sys.path.insert(0, _p)

import concourse.bass as bass
import concourse.bacc as bacc
import concourse.mybir as mybir
import concourse.tile as tile
from concourse.bass import ts
from concourse.bass_utils import run_bass_kernel_spmd

F32 = mybir.dt.float32
BF16 = mybir.dt.bfloat16
EXP = mybir.ActivationFunctionType.Exp

HID = 1024          # hidden dim
DS = 256            # per-core dim slice (4 heads x 64)
NT = 2048           # tokens per batch
HD = 64             # head dim
SCALE = HD ** -0.5
NKT = HID // 128    # hidden contraction tiles
NTOK = NT // 128    # token tiles of 128
NCH = NT // 256     # 256-wide token chunks

_NC_CACHE = {}
LABELS = {}


def _mm(nc, out, lhsT, rhs, start, stop, label=""):
    r = nc.tensor.matmul(out, lhsT, rhs, start=start, stop=stop)
    if label:
        try:
            LABELS[r.ins.name] = label
        except AttributeError:
            pass
    return r


def _build_nc(reps=1, upto="full", loop=None):
    """loop=N wraps one rep body in a hardware For_i(0, N) loop — used only
    for slope timing (amplifies signal without growing the NEFF)."""
    from contextlib import ExitStack, nullcontext

    nc = bacc.Bacc(num_swdge_queues=4)
    xqT = nc.declare_dram_parameter("xqT", [HID, NT], BF16, isOutput=False)
    xkT = nc.declare_dram_parameter("xkT", [HID, NT], BF16, isOutput=False)
    vT = nc.declare_dram_parameter("vT", [HID, NT], BF16, isOutput=False)
    wqT = nc.declare_dram_parameter("wqT", [128, NKT, DS], BF16, isOutput=False)
    wkT = nc.declare_dram_parameter("wkT", [128, NKT, DS], BF16, isOutput=False)
    wvT = nc.declare_dram_parameter("wvT", [128, NKT, DS], BF16, isOutput=False)
    wqb = nc.declare_dram_parameter("wqb", [128, 2], F32, isOutput=False)
    wkb = nc.declare_dram_parameter("wkb", [128, 2], F32, isOutput=False)
    projT = nc.declare_dram_parameter("projT", [128, 2, HID], BF16,
                                      isOutput=False)
    trimask4 = nc.declare_dram_parameter("trimask4", [128, 512], BF16,
                                         isOutput=False)
    ident = nc.declare_dram_parameter("ident", [128, 128], BF16,
                                      isOutput=False)
    y = nc.declare_dram_parameter("y", [NT, HID], BF16, isOutput=True)

    with tile.TileContext(nc) as tc, ExitStack() as ctx:
        ctx.enter_context(nc.allow_low_precision(
            reason="bf16 matmul datapath by design; fp32 psum accumulate"))
        pers = ctx.enter_context(tc.tile_pool(name="pers", bufs=1))

        KHT = [pers.tile([128, NT], BF16, tag=f"kht{i}", name=f"kht{i}")
               for i in range(2)]
        # Per-head zero-padded Q tiles (double-buffered by chunk parity).
        # bf16 matmuls with operands at base partition 64 fail at runtime on
        # this stack, so S uses K=128 from base partition 0 with the other
        # head's partitions zeroed in the rhs; the pad halves are zeroed once
        # here and never rewritten (drains only touch the data half).
        QHP = [[pers.tile([128, 256], BF16, tag=f"qh{p}{h}", name=f"qh{p}{h}")
                for h in range(4)] for p in range(2)]
        for p in range(2):
            for h in range(4):
                sub = h % 2
                nc.vector.memset(
                    QHP[p][h][64 * (1 - sub):64 * (1 - sub) + 64, :], 0.0)
        VH = [pers.tile([128, 4 * 65], BF16, tag=f"vh{m}", name=f"vh{m}")
              for m in range(NTOK)]

        wq_s = pers.tile([128, NKT, DS], BF16, tag="wq")
        wk_s = pers.tile([128, NKT, DS], BF16, tag="wk")
        wv_s = pers.tile([128, NKT, DS], BF16, tag="wv")
        pj_s = pers.tile([128, 2, HID], BF16, tag="pj")
        mk_s = pers.tile([128, 512], BF16, tag="mask")
        id_s = pers.tile([128, 128], BF16, tag="ident")
        qb_s = pers.tile([128, 2], F32, tag="wqb")
        kb_s = pers.tile([128, 2], F32, tag="wkb")

        # loop-timing mode: ch7's normalized xh lands in persistent tiles so
        # the NEXT For_i iteration can drain the tail overlapped with its
        # prologue (every iteration computes identical values, so the final
        # iteration's undrained tail leaves the correct y rows from the
        # previous drain).  Zeroed once so iteration 0's drain is finite.
        XH_P = None
        if loop is not None:
            XH_P = [[pers.tile([128, 128], BF16, tag=f"xhp{j}{hp}",
                               name=f"xhp{j}{hp}")
                     for hp in range(2)] for j in range(2)]
            for j in range(2):
                for hp in range(2):
                    nc.vector.memset(XH_P[j][hp][:], 0.0)

        for m in range(NTOK):
            vh3 = VH[m].rearrange("p (h w) -> p h w", w=65)
            nc.vector.memset(vh3[:, :, 64:65], 1.0)

        psp = ctx.enter_context(
            tc.tile_pool(name="psp", bufs=1, space=bass.MemorySpace.PSUM))
        xsp = ctx.enter_context(tc.tile_pool(name="xsp", bufs=1))
        qhp = ctx.enter_context(tc.tile_pool(name="qhp", bufs=8))
        ptp = ctx.enter_context(tc.tile_pool(name="ptp", bufs=18))
        asb = ctx.enter_context(tc.tile_pool(name="asb", bufs=4))
        xhp = ctx.enter_context(tc.tile_pool(name="xhp", bufs=4))
        avn = ctx.enter_context(tc.tile_pool(name="avn", bufs=4))
        ysb = ctx.enter_context(tc.tile_pool(name="ysb", bufs=2))

        # weight DMA order: what chunk 0 needs first (wq, qb) precedes the
        # bulk x streams; the rest rides behind xq
        nc.sync.dma_start(wq_s[:], wqT[:])
        nc.sync.dma_start(qb_s[:], wqb[:])

        first = True
        x3 = [xqT.rearrange("(kt p) n -> p kt n", p=128),
              xkT.rearrange("(kt p) n -> p kt n", p=128),
              vT.rearrange("(kt p) n -> p kt n", p=128)]
        XLOOK = 3                  # chunk prefetch depth
        _pending = {}
        _total_ch = reps * NCH

        def _ensure_load(g):
            if g in _pending or g >= _total_ch:
                return
            cg = g % NCH
            tls = []
            for ti, tag in enumerate(("xq", "xk", "xv")):
                tl = xsp.tile([128, NKT, 256], BF16, tag=tag, bufs=XLOOK + 1,
                              name=tag)
                nc.sync.dma_start(tl[:], x3[ti][:, :, ts(cg, 256)])
                tls.append(tl)
            _pending[g] = tls

        if loop is not None:
            # hardware-loop timing mode: weights load once before the loop
            assert reps == 1
            nc.sync.dma_start(wk_s[:], wkT[:])
            nc.sync.dma_start(kb_s[:], wkb[:])
            nc.sync.dma_start(wv_s[:], wvT[:])
            nc.sync.dma_start(mk_s[:], trimask4[:])
            nc.sync.dma_start(id_s[:], ident[:])
            nc.sync.dma_start(pj_s[:], projT[:])
            first = False
        tail = None
        for _rep in range(reps):
            _loop_cm = tc.For_i(0, loop, 1) if loop is not None else None
            if _loop_cm is not None:
                _loop_cm.__enter__()
            # per-chunk x streaming: one [128, NKT, 256] tile per (tensor,
            # chunk), prefetched XLOOK chunks ahead.  Rotating bufs give
            # cross-rep overlap (next rep's chunk-0 x loads while this rep
            # finishes) and keep SBUF small.
            if first:
                _ensure_load(_rep * NCH)
                nc.sync.dma_start(wk_s[:], wkT[:])
                nc.sync.dma_start(kb_s[:], wkb[:])
                nc.sync.dma_start(wv_s[:], wvT[:])
                nc.sync.dma_start(mk_s[:], trimask4[:])
                nc.sync.dma_start(id_s[:], ident[:])
                nc.sync.dma_start(pj_s[:], projT[:])
                first = False

            def tail_units(xhs, ch_t):
                # transpose / proj / store for a chunk whose normalized xh
                # tiles are ready.  Generator: driven interleaved into the
                # NEXT chunk's attention stream so PE keeps queued work while
                # ACT catches up on exp.
                tps = []
                for j in range(2):
                    tp = psp.tile([128, 256], BF16, tag="av", bufs=2,
                                  name="tp")
                    for hp in range(2):
                        nc.tensor.transpose(tp[:, ts(hp, 128)], xhs[j][hp][:],
                                            id_s[:])
                    tps.append(tp)
                yield
                for j in range(2):
                    m = 2 * ch_t + j
                    at = avn.tile([128, 256], BF16, tag="avn", name="avn")
                    nc.vector.tensor_copy(at[:], tps[j][:])
                    ys = ysb.tile([128, HID], BF16, tag="ys", name="ys")
                    for n2 in range(2):
                        yp = psp.tile([128, 512], F32, tag="pp", bufs=2,
                                      name="yp")
                        for hp in range(2):
                            _mm(nc, yp[:], at[:, ts(hp, 128)],
                                pj_s[:, hp, ts(n2, 512)],
                                start=(hp == 0), stop=(hp == 1),
                                label="proj")
                        nc.vector.tensor_copy(ys[:, ts(n2, 512)], yp[:])
                        yield
                    nc.sync.dma_start(y[ts(m, 128), :], ys[:])

            def proj_units(gp):
                # Q/K/V projections for global chunk gp.  Generator yielding
                # after each contraction step (~2 matmuls); driven interleaved
                # into the PREVIOUS chunk's attention.
                cp = gp % NCH
                for la in range(XLOOK + 1):
                    _ensure_load(gp + la)
                xq_t, xk_t, xv_t = _pending.pop(gp)
                qhn = QHP[cp % 2]
                ps = psp.tile([128, 512], F32, tag="pp", bufs=2, name="psQ")
                for kt in range(NKT):
                    for m in range(2):
                        _mm(nc, ps[:, ts(m, 256)], wq_s[:, kt, ts(m, 128)],
                            xq_t[:, kt, :],
                            start=(kt == 0 and m == 0),
                            stop=(kt == NKT - 1 and m == 1),
                            label="Qproj")
                    yield
                # qh bias-add on DVE (not ACT): keeps ACT exclusively on exp
                # so the exp stream never falls behind the PE's S matmuls
                for h in range(4):
                    m, sub = divmod(h, 2)
                    nc.vector.tensor_scalar_add(
                        qhn[h][64 * sub:64 * sub + 64, :],
                        ps[64 * sub:64 * sub + 64, ts(m, 256)],
                        qb_s[64 * sub:64 * sub + 64, m:m + 1])
                yield
                ps = psp.tile([128, 512], F32, tag="pp", bufs=2, name="psK")
                for kt in range(NKT):
                    for m in range(2):
                        _mm(nc, ps[:, ts(m, 256)], wk_s[:, kt, ts(m, 128)],
                            xk_t[:, kt, :],
                            start=(kt == 0 and m == 0),
                            stop=(kt == NKT - 1 and m == 1),
                            label="Kproj")
                    yield
                for m in range(2):
                    nc.vector.tensor_scalar_add(
                        KHT[m][:, ts(cp, 256)], ps[:, ts(m, 256)],
                        kb_s[:, m:m + 1])
                yield
                # wv_b is folded into the host epilogue (sum p = 1 after
                # normalize, so xh += wv_b  =>  y += wv_b @ proj_w.T)
                ps = psp.tile([128, 512], F32, tag="pp", bufs=2, name="psV")
                for kt in range(NKT):
                    for m2 in range(2):
                        _mm(nc, ps[:, ts(m2, 256)],
                            xv_t[:, kt, ts(m2, 128)],
                            wv_s[:, kt, :],
                            start=(kt == 0 and m2 == 0),
                            stop=(kt == NKT - 1 and m2 == 1),
                            label="Vproj")
                    yield
                for m2 in range(2):
                    vh3 = VH[2 * cp + m2].rearrange("p (h w) -> p h w", w=65)
                    nc.vector.tensor_copy(
                        vh3[:, :, 0:64],
                        ps[:, ts(m2, 256)].rearrange("p (h w) -> p h w", w=64))

            if _rep == 0:
                # prologue: in loop mode, drain the previous iteration's
                # tail first -- it has no x dependence, so its PE/DVE/DMA
                # work fills the chunk-0 x DMA wait; then chunk 0's
                # projections run standalone
                if loop is not None:
                    for _ in tail_units(XH_P, NCH - 1):
                        pass
                for _ in proj_units(_rep * NCH):
                    pass

            for ch in range(NCH):
                g = _rep * NCH + ch
                qh = QHP[ch % 2]
                # interleave units: next chunk's projections first (so the
                # qh/kht DVE writes the next chunk's S(0) needs queue ahead
                # of the tail's copies), then the previous chunk's tail
                units = []
                if g + 1 < _total_ch:
                    units.append(proj_units(g + 1))
                    # a few proj matmuls ahead of the tail's PE transposes:
                    # the transposes wait on the previous chunk's norm (DVE),
                    # and PE is in-order, so give DVE a head start
                    for _ in range(3):
                        try:
                            next(units[0])
                        except StopIteration:
                            break
                if tail is not None:
                    tg = tail_units(*tail)
                    next(tg)  # allocate tp psum tiles before av accumulators
                    units.append(tg)
                    tail = None

                def step(n):
                    for _ in range(n):
                        if not units:
                            return
                        try:
                            next(units[0])
                        except StopIteration:
                            units.pop(0)

                # ---- attention: S/exp/AV streamed over k-tiles, with a
                # one-step skew so AV(i) is emitted after S(i+1); ~31
                # interleave units paced evenly across the k-tiles ----
                nkt = 2 * ch + 2
                NUNITS = 31
                _done = [0]

                def pace(i):
                    tgt = ((i + 1) * NUNITS) // nkt
                    step(tgt - _done[0])
                    _done[0] = tgt
                av = [psp.tile([128, 260], F32, tag="av", bufs=2,
                               name=f"av{hp}") for hp in range(2)]
                pts = []

                def emit_av(i):
                    cs = max(0, 128 * (i - 2 * ch))
                    for j in range(2):
                        if i > 2 * ch + j:
                            continue
                        off = 128 * j - cs
                        for h in range(4):
                            hp, s2 = divmod(h, 2)
                            _mm(nc,
                                av[hp][:, 130 * j + 65 * s2:
                                       130 * j + 65 * s2 + 65],
                                pts[i][:, 256 * h + off:256 * h + off + 128],
                                VH[i][:, 65 * h:65 * h + 65],
                                start=(i == 0 and j == 0 and s2 == 0),
                                stop=(i == 2 * ch + 1 and j == 1
                                      and s2 == 1), label="AV")

                for i in range(nkt):
                    pace(i)  # queue PE work ahead of S so ACT keeps pace
                    d = i - 2 * ch
                    cs = 128 * d if d >= 0 else 0
                    w = 256 - cs
                    sp = psp.tile([128, 1024], F32, tag="sp", bufs=2,
                                  name="sp")
                    for h in range(4):
                        _mm(nc, sp[:, 256 * h:256 * h + w],
                            KHT[h // 2][:, ts(i, 128)], qh[h][:, cs:256],
                            start=(h % 2 == 0), stop=(h % 2 == 1), label="S")
                    pt = ptp.tile([128, 1024], BF16, tag="pt", name="pt")
                    sp4 = sp.rearrange("p (h w) -> p h w", w=256)
                    pt4 = pt.rearrange("p (h w) -> p h w", w=256)
                    nc.scalar.activation(pt4[:, :, 0:w], sp4[:, :, 0:w],
                                         EXP, scale=SCALE)
                    if d >= 0:
                        nc.vector.tensor_mul(
                            pt4[:, :, 0:128], pt4[:, :, 0:128],
                            mk_s[:].rearrange("p (h w) -> p h w", w=128))
                    pts.append(pt)
                    if i > 2:
                        emit_av(i - 3)
                for ii in range(max(0, nkt - 3), nkt):
                    step(2)  # keep PE fed while the last exps finish
                    emit_av(ii)
                step(10 ** 6)  # drain remaining interleave units

                # ---- normalization (DVE): overlaps next chunk's attention
                xhs = []
                for j in range(2):
                    xhj = []
                    for hp in range(2):
                        av4 = av[hp].rearrange("p (j s w) -> p j s w",
                                               j=2, s=2)
                        rec2 = asb.tile([128, 2], F32, tag="rec", name="rec")
                        nc.vector.reciprocal(
                            rec2[:].rearrange("p (s w) -> p s w", w=1),
                            av4[:, j, :, 64:65])
                        if loop is not None and ch == NCH - 1:
                            # persistent tiles: drained at the start of the
                            # NEXT For_i iteration (values are identical
                            # every iteration, so the final y rows are
                            # already correct from the previous drain)
                            xh = XH_P[j][hp]
                        else:
                            xh = xhp.tile([128, 128], BF16, tag="xh", bufs=8,
                                          name="xh")
                        for s2 in range(2):
                            nc.vector.tensor_scalar_mul(
                                xh[:, 64 * s2:64 * s2 + 64],
                                av4[:, j, s2, 0:64], rec2[:, s2:s2 + 1])
                        xhj.append(xh)
                    xhs.append(xhj)
                tail = (xhs, ch)
            if loop is not None:
                # ch7's tail drains at the next iteration's prologue
                tail = None
            if _loop_cm is not None:
                _loop_cm.__exit__(None, None, None)
        if tail is not None:
            for _ in tail_units(*tail):
                pass

    nc.compile()
    return nc


def _get_nc():
    if "nc" not in _NC_CACHE:
        _NC_CACHE["nc"] = _build_nc()
    return _NC_CACHE["nc"]


def make_in_maps(q, k, v, qpos, kpos, mask, wq_w, wq_b, wk_w, wk_b, wv_w, wv_b,
                 proj_w, proj_b):
    f32 = np.float32
    bf16 = mybir.dt.np(BF16)
    q = np.asarray(q, f32); k = np.asarray(k, f32); v = np.asarray(v, f32)
    qpos = np.asarray(qpos, f32); kpos = np.asarray(kpos, f32)
    wq_w = np.asarray(wq_w, f32); wk_w = np.asarray(wk_w, f32)
    wv_w = np.asarray(wv_w, f32); proj_w = np.asarray(proj_w, f32)
    wq_b = np.asarray(wq_b, f32); wk_b = np.asarray(wk_b, f32)
    wv_b = np.asarray(wv_b, f32)

    m2 = np.asarray(mask).reshape(NT, NT)
    # pt layout is [k_local, q_local]; valid (unmasked) = 1.0
    pat = (~m2[0:128, 0:128]).astype(f32).T
    trimask4 = np.concatenate([pat] * 4, axis=1).astype(bf16)
    ident = np.eye(128, dtype=f32).astype(bf16)

    actT = {}
    for b in range(2):
        actT[("xq", b)] = np.ascontiguousarray((q[b] + qpos[b]).T).astype(bf16)
        actT[("xk", b)] = np.ascontiguousarray((k[b] + kpos[b]).T).astype(bf16)
        actT[("v", b)] = np.ascontiguousarray(v[b].T).astype(bf16)

    in_maps = []
    for c in range(8):
        b, g = divmod(c, 4)
        sl = slice(DS * g, DS * (g + 1))
        in_maps.append({
            "xqT": actT[("xq", b)], "xkT": actT[("xk", b)],
            "vT": actT[("v", b)],
            "wqT": np.ascontiguousarray(
                wq_w[sl, :].T.reshape(NKT, 128, DS).transpose(1, 0, 2)
            ).astype(bf16),
            "wkT": np.ascontiguousarray(
                wk_w[sl, :].T.reshape(NKT, 128, DS).transpose(1, 0, 2)
            ).astype(bf16),
            "wvT": np.ascontiguousarray(
                wv_w[sl, :].T.reshape(NKT, 128, DS).transpose(1, 0, 2)
            ).astype(bf16),
            "wqb": np.ascontiguousarray(wq_b[sl].reshape(2, 128).T),
            "wkb": np.ascontiguousarray(wk_b[sl].reshape(2, 128).T),
            "projT": np.ascontiguousarray(
                proj_w[:, sl].T.reshape(2, 128, HID).transpose(1, 0, 2)
            ).astype(bf16),
            "trimask4": trimask4,
            "ident": ident,
        })
    return in_maps


def kernel(q, k, v, qpos, kpos, mask, wq_w, wq_b, wk_w, wk_b, wv_w, wv_b,
           proj_w, proj_b, _trace=False):
    import time as _time
    nc = _get_nc()
    in_maps = make_in_maps(q, k, v, qpos, kpos, mask, wq_w, wq_b, wk_w, wk_b,
                           wv_w, wv_b, proj_w, proj_b)
    try:
        res = run_bass_kernel_spmd(nc, in_maps, list(range(8)), trace=_trace)
    except Exception:
        # transient device faults (mesh desync / NRT exec-unit errors) have
        # been observed to recover on retry; one guarded attempt
        _time.sleep(15)
        res = run_bass_kernel_spmd(nc, in_maps, list(range(8)), trace=_trace)
    if _trace:
        kernel._last_results = res
    out = np.zeros((2, NT, HID), np.float32)
    for c in range(8):
        out[c // 4] += np.asarray(res.results[c]["y"], np.float32)
    # V-bias epilogue: after softmax-normalization sum(p)=1, so the V bias
    # adds wv_b to every attention output row => y += wv_b @ proj_w.T.
    epi = (np.asarray(wv_b, np.float64) @ np.asarray(proj_w, np.float64).T
           + np.asarray(proj_b, np.float64))
    out += epi.astype(np.float32)[None, None, :]
    return out

